# revision 31
# baseline (speedup 1.0000x reference)
"""Tensor-parallel GQA multi-head attention (RoPE + causal softmax) for 8 trn2 cores.

Sharding: 8 cores = 2 batches x 4 head-groups. Core c handles batch c//4 and
q-heads [8g, 8g+8) / kv-heads {2g, 2g+1} where g = c%4. Projections and
flash-style causal attention are interleaved per 512-token slab to keep the
tensor engine dense. After each slab the normalized attention outputs (bf16,
512KB) are AllGathered within the batch group, so every core sees all 32
heads' features and computes its own 512-column slice of the output
projection locally - no reduce needed. Host stitches the column shards.
"""

import sys

sys.path.insert(0, "/opt/trn_rl_repo")

import numpy as np

import concourse.bass as bass
import concourse.bacc as bacc
import concourse.mybir as mybir
from concourse import tile
from concourse.bass_utils import run_bass_kernel_spmd

B, S, D = 2, 2048, 2048
N_HEADS, N_KV, HD = 32, 8, 64
NCORES = 8
NG = 4  # head groups = cores per batch
QH = 8  # q-heads per core
KVH = 2  # kv-heads per core
FQ = QH * HD  # 512
FKV = 2 * KVH * HD  # 256 (K then V)
SCALE = 1.0 / 8.0  # 1/sqrt(HD)
MASK_NEG = -30000.0

QTILE = 512
KTILE = 128
NSLAB = S // QTILE  # 4
ND = D // 128  # 16 contraction chunks

F32 = mybir.dt.float32
EXP = mybir.ActivationFunctionType.Exp

BF16 = mybir.dt.bfloat16
MMD = BF16  # matmul operand dtype

GROUPS = [[0, 1, 2, 3], [4, 5, 6, 7]]


def _build_kernel(tc, io):
    nc = tc.nc
    xT, wq, wkv, wo = io["xT"], io["wq"], io["wkv"], io["wo"]
    cos2, sin2s, trimask, sel = io["cos2"], io["sin2s"], io["trimask"], io["sel"]
    out_full = io["out"]
    single = bool(io.get("single"))

    # ---- SBUF pools ----
    const = tc.alloc_tile_pool(name="const", bufs=1)
    dram = tc.alloc_tile_pool(name="dram", bufs=1, space="DRAM")
    qkv = tc.alloc_tile_pool(name="qkv", bufs=1)              # QT, KK
    vap = tc.alloc_tile_pool(name="vap", bufs=1)              # V chunks
    tables = tc.alloc_tile_pool(name="tables", bufs=1)
    wA = tc.alloc_tile_pool(name="wA", bufs=1)                # proj weights
    wop = tc.alloc_tile_pool(name="wop", bufs=1, side="right")  # wo weights
    xq_pool = tc.alloc_tile_pool(name="xq", bufs=1)
    rp = tc.alloc_tile_pool(name="rope", bufs=2)
    aop = tc.alloc_tile_pool(name="aop", bufs=1, side="right")
    aog = tc.alloc_tile_pool(name="aog", bufs=1, side="right")
    fin = tc.alloc_tile_pool(name="fin", bufs=1, side="right")
    pexp = tc.alloc_tile_pool(name="pexp", bufs=3)
    evac = tc.alloc_tile_pool(name="evac", bufs=2)

    # ---- PSUM pools: exactly 8 banks ----
    psS = tc.alloc_tile_pool(name="psS", bufs=2, space="PSUM")  # scores, 2x2 banks
    psO = tc.alloc_tile_pool(name="psO", bufs=1, space="PSUM")  # oA+oB, 2 banks
    psX = tc.alloc_tile_pool(name="psX", bufs=2, space="PSUM")  # proj/wo/bc, 2 banks

    # ---- constants ----
    trimask_t = const.tile([KTILE, KTILE], F32)
    nc.sync.dma_start(trimask_t[:], trimask[:])
    sel_t = const.tile([QH, FQ], MMD)
    nc.sync.dma_start(sel_t[:], sel[:])
    ident = const.tile([128, 64], F32)
    nc.gpsimd.memset(ident[:], 0.0)
    for p in (0, 64):
        nc.gpsimd.affine_select(
            out=ident[p:p + 64, :], in_=ident[p:p + 64, :],
            compare_op=mybir.AluOpType.not_equal,
            fill=1.0, base=0, pattern=[[-1, 64]], channel_multiplier=1,
        )
    ones_col = const.tile([128, 1], F32)
    nc.vector.memset(ones_col[:], 1.0)
    prot_t = const.tile([128, 128], MMD)
    nc.sync.dma_start(prot_t[:], io["prot"][:])

    cos2_t = tables.tile([128, S], MMD)
    nc.sync.dma_start(cos2_t[:], cos2[:])
    sin2s_t = tables.tile([128, S], MMD)
    nc.sync.dma_start(sin2s_t[:], sin2s[:])

    QT = [qkv.tile([128, S], MMD, name=f"qt{t}") for t in range(4)]
    KK = qkv.tile([128, S], MMD)  # rows 0:64 K^T kv0, 64:128 K^T kv1
    AO = [aop.tile([128, S], MMD, name=f"ao{t}") for t in range(4)]
    denom = aop.tile([QH, S], MMD)

    ag_in = [dram.tile([FQ, QTILE], MMD, name=f"agin{j}")
             for j in range(NSLAB)]
    ag_out = [dram.tile([4 * FQ, QTILE], MMD, name=f"agout{j}")
              for j in range(NSLAB)]

    def load_xslab(j):
        xts = []
        for k in range(ND):
            xt = xq_pool.tile([128, QTILE], MMD, name="xt", tag=f"xt{k}")
            nc.sync.dma_start(
                xt[:], xT[k * 128:(k + 1) * 128, j * QTILE:(j + 1) * QTILE])
            xts.append(xt)
        return xts

    xts = load_xslab(0)  # first activations slab before the weight bulk
    Wt = {}
    for k in range(ND):
        wq_t = wA.tile([128, FQ], MMD, name=f"wq{k}")
        nc.sync.dma_start(wq_t[:], wq[k * 128:(k + 1) * 128, :])
        wkv_t = wA.tile([128, FKV], MMD, name=f"wkv{k}")
        nc.sync.dma_start(wkv_t[:], wkv[k * 128:(k + 1) * 128, :])
        for f in range(4):
            Wt[f, k] = wq_t[:, f * 128:(f + 1) * 128]
        Wt[4, k] = wkv_t[:, 0:128]
        Wt[5, k] = wkv_t[:, 128:256]
    WO = {}
    for fc in range(ND):
        wo_t = wop.tile([128, FQ], MMD, name=f"wot{fc}")
        nc.sync.dma_start(wo_t[:], wo[fc * 128:(fc + 1) * 128, :])
        for cc in range(4):
            WO[fc, cc] = wo_t[:, cc * 128:(cc + 1) * 128]

    VA = {}

    # ============ per-slab projections + RoPE + V transpose ============
    def proj_group(j, f, xts):
        qs = slice(j * QTILE, (j + 1) * QTILE)
        if True:
            ps = psX.tile([128, QTILE], F32, name="psA", tag="x")
            for k in range(ND):
                nc.tensor.matmul(ps[:], Wt[f, k], xts[k][:],
                                 start=(k == 0), stop=(k == ND - 1))
            if f < 5:
                # QT tiles 0..3 and KK: evacuate then RoPE the slab in place.
                # rotate_half (with signs) via a signed-permutation matmul.
                dst = QT[f] if f < 4 else KK
                nc.scalar.copy(dst[:, qs], ps[:])
                rot = psX.tile([128, QTILE], F32, name="rot", tag="x")
                nc.tensor.matmul(rot[:], prot_t[:], dst[:, qs],
                                 start=True, stop=True)
                t1 = rp.tile([128, QTILE], F32, name="t1", tag="t1")
                nc.vector.tensor_mul(t1[:], dst[:, qs], cos2_t[:, qs])
                t2 = rp.tile([128, QTILE], F32, name="t2", tag="t2")
                nc.vector.tensor_mul(t2[:], rot[:], sin2s_t[:, qs])
                nc.vector.tensor_add(dst[:, qs], t1[:], t2[:])
            else:
                # V^T slab: evacuate then transpose 128-chunks into [k, d]
                vv = rp.tile([128, QTILE], F32, name="vv", tag="vv")
                nc.scalar.copy(vv[:], ps[:])
                for kv in range(KVH):
                    for c in range(4):
                        i = 4 * j + c
                        tp = psX.tile([128, QTILE], F32, name="tp", tag="x")
                        nc.tensor.matmul(tp[:, 0:HD],
                                         vv[kv * 64:(kv + 1) * 64,
                                            c * 128:(c + 1) * 128],
                                         ident[kv * 64:(kv + 1) * 64, :],
                                         is_transpose=True, start=True,
                                         stop=True)
                        va = vap.tile([128, HD + 1], MMD, name=f"va{kv}_{i}")
                        nc.scalar.copy(va[:, 0:HD], tp[:, 0:HD])
                        nc.scalar.copy(va[:, HD:HD + 1], ones_col[:])
                        VA[kv, i] = va

    # ==== attention for one pair-tile t: scores -> exp -> PV, softpiped ====
    def score_exp(j, t, i):
        r = i - 4 * j
        off = max(r, 0) * KTILE
        ks = slice(i * KTILE, (i + 1) * KTILE)
        qv = slice(j * QTILE + off, (j + 1) * QTILE)
        sAB = psS.tile([KTILE, 2 * QTILE], F32, name="sAB", tag="sAB")
        nc.tensor.matmul(sAB[:, off:QTILE], KK[0:64, ks], QT[t][0:64, qv],
                         start=True, stop=True, tile_position=(0, 0))
        nc.tensor.matmul(sAB[:, QTILE + off:], KK[64:128, ks],
                         QT[t][64:128, qv],
                         start=True, stop=True, tile_position=(64, 0))
        pAB = pexp.tile([KTILE, 2 * QTILE], MMD, name="pAB", tag="pAB")
        if r >= 0:
            nc.vector.tensor_add(sAB[:, off:off + KTILE],
                                 sAB[:, off:off + KTILE], trimask_t[:])
            nc.vector.tensor_add(
                sAB[:, QTILE + off:QTILE + off + KTILE],
                sAB[:, QTILE + off:QTILE + off + KTILE], trimask_t[:])
            nc.scalar.activation(pAB[:, off:QTILE], sAB[:, off:QTILE],
                                 EXP, scale=SCALE)
            nc.scalar.activation(pAB[:, QTILE + off:],
                                 sAB[:, QTILE + off:], EXP, scale=SCALE)
        else:
            nc.scalar.activation(pAB[:], sAB[:], EXP, scale=SCALE)
        return pAB

    def attention_t(j, t):
        qs = slice(j * QTILE, (j + 1) * QTILE)
        oA = psO.tile([HD + 1, QTILE], F32, name="oA", tag="oA")
        oB = psO.tile([HD + 1, QTILE], F32, name="oB", tag="oB")
        nkt = 4 * j + 4

        def pv(i, pAB):
            off = max(i - 4 * j, 0) * KTILE
            nc.tensor.matmul(oA[:, off:], VA[0, i][:], pAB[:, off:QTILE],
                             start=(i == 0), stop=(i == nkt - 1))
            nc.tensor.matmul(oB[:, off:], VA[1, i][:], pAB[:, QTILE + off:],
                             start=(i == 0), stop=(i == nkt - 1))

        prev = score_exp(j, t, 0)
        for i in range(1, nkt):
            cur = score_exp(j, t, i)
            pv(i - 1, prev)
            prev = cur
        pv(nkt - 1, prev)
        # evacuate: rows 0:64 outT, row 64 denominator
        tA = evac.tile([HD + 1, QTILE], MMD, name="tA", tag="tA")
        tB = evac.tile([HD + 1, QTILE], MMD, name="tB", tag="tB")
        nc.vector.tensor_copy(tA[:], oA[:])
        nc.vector.tensor_copy(tB[:], oB[:])
        nc.gpsimd.dma_start(AO[t][0:64, qs], tA[0:64, :])
        nc.gpsimd.dma_start(AO[t][64:128, qs], tB[0:64, :])
        nc.gpsimd.dma_start(denom[t:t + 1, qs], tA[64:65, :])
        nc.gpsimd.dma_start(denom[t + 4:t + 5, qs], tB[64:65, :])

    # normalize AO, stage into DRAM, and kick the head-gather AllGather
    def stage_slab(j):
        qs = slice(j * QTILE, (j + 1) * QTILE)
        dR = fin.tile([QH, QTILE], F32, name="dR", tag="dR")
        nc.vector.reciprocal(dR[:], denom[:, qs])
        dRb = fin.tile([QH, QTILE], MMD, name="dRb", tag="dRb")
        nc.vector.tensor_copy(dRb[:], dR[:])
        for t in range(4):
            bc = psX.tile([128, QTILE], F32, name="bc", tag="x")
            nc.tensor.matmul(bc[:], sel_t[:, t * 128:(t + 1) * 128],
                             dRb[:], start=True, stop=True)
            nc.vector.tensor_mul(AO[t][:, qs], AO[t][:, qs], bc[:])
            nc.gpsimd.dma_start(ag_in[j][128 * t:128 * (t + 1), :],
                                AO[t][:, qs])
        if single or io.get("mock_cc"):
            nc.gpsimd.dma_start(ag_out[j][0:FQ, :], ag_in[j][:])
        else:
            nc.gpsimd.collective_compute(
                "AllGather",
                mybir.AluOpType.bypass,
                replica_groups=GROUPS,
                ins=[ag_in[j][:]],
                outs=[ag_out[j][:]],
            )

    # output projection: this core's 512-column slice for slab jj's tokens
    def load_aog(jj):
        AOG = []
        for fc in range(ND):
            ag = aog.tile([128, QTILE], MMD, name=f"aog{fc}", tag=f"aog{fc}")
            nc.gpsimd.dma_start(ag[:], ag_out[jj][128 * fc:128 * (fc + 1), :])
            AOG.append(ag)
        return AOG

    def wo_chunk(jj, cc, AOG):
        ps = psX.tile([128, QTILE], F32, name="psWo", tag="x")
        for fc in range(ND):
            nc.tensor.matmul(ps[:], WO[fc, cc], AOG[fc][:],
                             start=(fc == 0), stop=(fc == ND - 1))
        og = evac.tile([128, QTILE], F32, name="og", tag="og")
        nc.vector.tensor_copy(og[:], ps[:])
        nc.gpsimd.dma_start(
            out_full[cc * 128:(cc + 1) * 128,
                     jj * QTILE:(jj + 1) * QTILE], og[:])

    # ---- schedule: softpiped attention with proj/wo interleaved per t ----
    PROJ_SCHED = {0: (0, 1), 1: (2, 3), 2: (4,), 3: (5,)}
    for f in range(6):
        proj_group(0, f, xts)
    if NSLAB > 1:
        xts = load_xslab(1)
    AOG = None
    for j in range(NSLAB):
        for t in range(4):
            attention_t(j, t)
            if t == 3:
                # kick normalize + AllGather the moment attention finishes,
                # ahead of the interleaved proj/wo filler work
                stage_slab(j)
            if j > 0:
                # defer wo chunks so they never head-of-line block the PE
                # queue while their AllGather is still in flight
                if t == 0:
                    AOG = load_aog(j - 1)
                elif t >= 2:
                    wo_chunk(j - 1, t - 2, AOG)
            if j + 1 < NSLAB:
                for f in PROJ_SCHED[t]:
                    proj_group(j + 1, f, xts)
        if j > 0:
            wo_chunk(j - 1, 2, AOG)
            wo_chunk(j - 1, 3, AOG)
        if j + 2 < NSLAB:
            xts = load_xslab(j + 2)
    AOG = load_aog(NSLAB - 1)
    for cc in range(4):
        wo_chunk(NSLAB - 1, cc, AOG)

    psX.release()
    psO.release()
    psS.release()
    evac.release()
    pexp.release()
    fin.release()
    aog.release()
    aop.release()
    rp.release()
    xq_pool.release()
    wop.release()
    wA.release()
    tables.release()
    vap.release()
    qkv.release()
    dram.release()
    const.release()


import os


def _build(single=False):
    nc = bacc.Bacc("TRN2", target_bir_lowering=False, debug=False,
                   num_devices=1 if single else NCORES)
    io = {
        "xT": nc.dram_tensor("xT", [D, S], BF16, kind="ExternalInput").ap(),
        "wq": nc.dram_tensor("wq", [D, FQ], BF16, kind="ExternalInput").ap(),
        "wkv": nc.dram_tensor("wkv", [D, FKV], BF16, kind="ExternalInput").ap(),
        "wo": nc.dram_tensor("wo", [D, FQ], BF16, kind="ExternalInput").ap(),
        "cos2": nc.dram_tensor("cos2", [128, S], BF16, kind="ExternalInput").ap(),
        "sin2s": nc.dram_tensor("sin2s", [128, S], BF16, kind="ExternalInput").ap(),
        "trimask": nc.dram_tensor("trimask", [KTILE, KTILE], F32,
                                  kind="ExternalInput").ap(),
        "sel": nc.dram_tensor("sel", [QH, FQ], BF16, kind="ExternalInput").ap(),
        "prot": nc.dram_tensor("prot", [128, 128], BF16,
                               kind="ExternalInput").ap(),
        "out": nc.dram_tensor("out", [FQ, S], F32,
                              kind="ExternalOutput").ap(),
    }
    io["single"] = single
    io["mock_cc"] = bool(os.environ.get("K_MOCK_CC"))
    with tile.TileContext(nc) as tc:
        _build_kernel(tc, io)
    nc.compile()
    return nc


_CACHE = {}


def _get_program():
    if "nc" not in _CACHE:
        _CACHE["nc"] = _build()
    return _CACHE["nc"]


def _host_inputs(x, wq, wk, wv, wo):
    x = np.ascontiguousarray(x, np.float32)
    inv = 1.0 / (10000.0 ** (np.arange(0, HD, 2, dtype=np.float64) / HD))
    pos = np.arange(S, dtype=np.float64)
    freqs = np.outer(pos, inv)  # [S, 32]
    emb = np.concatenate([freqs, freqs], axis=1)  # [S, 64]
    cos = np.cos(emb).T.astype(np.float32)  # [64, S]
    sin = np.sin(emb).T.astype(np.float32)
    cos2 = np.concatenate([cos, cos], axis=0)  # [128, S]
    sin2s = np.concatenate([sin, sin], axis=0)  # signs live in the rot matmul

    # signed rotate-half permutation R (per 64-row head block):
    # R[r, r+32] = -1 for r in [0,32), R[r, r-32] = +1 for r in [32,64)
    R = np.zeros((128, 128), np.float32)
    for h in range(2):
        b = h * 64
        for r in range(32):
            R[b + r, b + r + 32] = -1.0
            R[b + r + 32, b + r] = 1.0
    prot = np.ascontiguousarray(R.T)  # lhsT convention: out = lhsT.T @ rhs

    kk, qq = np.meshgrid(np.arange(KTILE), np.arange(KTILE), indexing="ij")
    trimask = np.where(kk <= qq, 0.0, MASK_NEG).astype(np.float32)

    # attn_outT row layout per pair-tile t: rows 0:64 head t, 64:128 head t+4
    sel = np.zeros((QH, FQ), np.float32)
    for t in range(4):
        sel[t, t * 128:t * 128 + 64] = 1.0
        sel[t + 4, t * 128 + 64:(t + 1) * 128] = 1.0

    import ml_dtypes
    bf16 = ml_dtypes.bfloat16
    cos2 = cos2.astype(bf16)
    sin2s = sin2s.astype(bf16)
    sel = sel.astype(bf16)
    prot = prot.astype(bf16)
    xT = [np.ascontiguousarray(x[b].T.astype(bf16)) for b in range(B)]
    # wo rows in ag_out feature order: source rank r major, then pair-tile t,
    # then head r*8+t rows followed by head r*8+t+4 rows
    wrows = []
    for r in range(NG):
        for t in range(4):
            for h in (8 * r + t, 8 * r + t + 4):
                wrows.append(wo[h * HD:(h + 1) * HD, :])
    wo_r = np.concatenate(wrows, axis=0)  # [D, D] reordered rows
    in_maps = []
    for c in range(NCORES):
        b, g = c // NG, c % NG
        # pair-tile column order: heads (t, t+4) interleaved per 128-col tile
        qcols = []
        for t in range(4):
            for h in (8 * g + t, 8 * g + t + 4):
                qcols.append(wq[:, h * HD:(h + 1) * HD])
        wq_p = np.ascontiguousarray(np.concatenate(qcols, axis=1).astype(bf16))
        kv0 = 2 * g
        wkv_p = np.ascontiguousarray(np.concatenate(
            [wk[:, kv0 * HD:(kv0 + 2) * HD], wv[:, kv0 * HD:(kv0 + 2) * HD]],
            axis=1).astype(bf16))
        wo_p = np.ascontiguousarray(
            wo_r[:, g * FQ:(g + 1) * FQ].astype(bf16))
        in_maps.append({
            "xT": xT[b], "wq": wq_p, "wkv": wkv_p, "wo": wo_p,
            "cos2": cos2, "sin2s": sin2s, "trimask": trimask, "sel": sel,
            "prot": prot,
        })
    return in_maps


def run(x, wq, wk, wv, wo, trace=False, **trace_kwargs):
    nc = _get_program()
    in_maps = _host_inputs(x, wq, wk, wv, wo)
    res = run_bass_kernel_spmd(nc, in_maps, list(range(NCORES)),
                               trace=trace, **trace_kwargs)
    out = np.empty((B, S, D), np.float32)
    for c in range(NCORES):
        b, g = c // NG, c % NG
        shard = res.results[c]["out"]  # [512 cols, S tokens]
        out[b, :, g * FQ:(g + 1) * FQ] = shard.T
    return out, res


def kernel(x, wq, wk, wv, wo):
    out, _ = run(x, wq, wk, wv, wo)
    return out.astype(np.float32)
